# revision 3
# baseline (speedup 1.0000x reference)
"""GQA cross-attention kernel for 8 trn2 NeuronCores.

Problem: q [2, 2048, 32, 128] fp32, kv [2, 2048, 2, 8, 128] fp32
         -> softmax(q @ k^T / sqrt(128)) @ v  -> [2, 2048, 32, 128]

Sharding: 64 (batch, head) units over 8 cores: core c gets batch c//4,
q-heads [8*(c%4), 8*(c%4)+8) and kv-heads [2*(c%4), 2*(c%4)+2).

Device layout (host pre-transposes, free):
  qT  [8, 128, 2048]  = q head-major, D on partitions
  kT  [2, 128, 2048]  = k head-major, D on partitions
  vt  [2, 128, 2048]  = v tiled: vt[i, p, t*128+d] = v[t*128+p, d]
  oT  [8, 128, 2048]  = output O^T per head (host transposes back)

Per (head, 512-wide q block): stream 16 k-tiles of 128:
  S^T tile = K_tile^T . Q_block   (fp32r matmul, [128 sk, 512 sq] PSUM)
  P = exp(scale * S^T)            (ScalarE, PSUM->SBUF; scores ~N(0,1) so
                                   no max subtraction needed in fp32)
  l += ones^T . P                 (row sums via PE accumulation, [1, 512])
  O^T += V_tile^T . P             (PSUM accumulation, [128 d, 512 sq])
then epilogue: recip(l) -> partition broadcast -> multiply+evacuate -> DMA.
"""

import math

import numpy as np

import concourse.bass as bass
import concourse.mybir as mybir
import concourse.tile as tile
from concourse import bacc
from concourse.bass_utils import run_bass_kernel_spmd

F32 = mybir.dt.float32
F32R = mybir.dt.float32r
EXP = mybir.ActivationFunctionType.Exp

B, SQ, SK, H, HKV, D = 2, 2048, 2048, 32, 8, 128
N_CORES = 8
H_PER_CORE = H * B // N_CORES  # 8
KV_PER_CORE = HKV * B // N_CORES  # 2
SCALE = 1.0 / math.sqrt(D)


def build_nc(
    n_heads=H_PER_CORE,
    n_kv=KV_PER_CORE,
    sq=SQ,
    sk=SK,
    sq_blk=512,
    pair=2,
):
    """Build the SPMD Bass program (identical on all cores)."""
    assert n_heads % n_kv == 0
    heads_per_kv = n_heads // n_kv
    sk_tiles = sk // 128
    sq_blocks = sq // sq_blk
    assert sk_tiles % pair == 0

    nc = bacc.Bacc("TRN2", target_bir_lowering=False, debug=False)

    qT = nc.dram_tensor("qT", [n_heads, D, sq], F32R, kind="ExternalInput")
    kT = nc.dram_tensor("kT", [n_kv, D, sk], F32R, kind="ExternalInput")
    vt = nc.dram_tensor("vt", [n_kv, 128, sk_tiles * D], F32R, kind="ExternalInput")
    ones = nc.dram_tensor("ones", [128, 1], F32R, kind="ExternalInput")
    oT = nc.dram_tensor("oT", [n_heads, D, sq], F32, kind="ExternalOutput")

    with tile.TileContext(nc) as tc:
        with (
            tc.tile_pool(name="inp", bufs=1) as inp_pool,
            tc.tile_pool(name="ppool", bufs=3) as ppool,
            tc.tile_pool(name="rpool", bufs=2) as rpool,
            tc.tile_pool(name="bpool", bufs=2) as bpool,
            tc.tile_pool(name="outp", bufs=3) as outp,
            tc.tile_pool(name="spsum", bufs=2, space="PSUM") as spsum,
            tc.tile_pool(name="opsum", bufs=2, space="PSUM") as opsum,
            tc.tile_pool(name="lpsum", bufs=2, space="PSUM") as lpsum,
        ):
            ones_sb = inp_pool.tile([128, 1], F32R, tag="ones", name="ones_sb")
            nc.sync.dma_start(ones_sb[:], ones[:])

            q_sb = [None] * n_heads
            k_sb = [None] * n_kv
            v_sb = [None] * n_kv
            # DMA order: kv group 0 + its q heads first so compute starts early.
            for g in range(n_kv):
                k_sb[g] = inp_pool.tile([D, sk], F32R, tag=f"k{g}", name=f"k_sb{g}")
                v_sb[g] = inp_pool.tile([128, sk_tiles * D], F32R, tag=f"v{g}", name=f"v_sb{g}")
                nc.sync.dma_start(k_sb[g][:], kT[g])
                nc.sync.dma_start(v_sb[g][:], vt[g])
                for hh in range(heads_per_kv):
                    h = g * heads_per_kv + hh
                    q_sb[h] = inp_pool.tile([D, sq], F32R, tag=f"q{h}", name=f"q_sb{h}")
                    nc.sync.dma_start(q_sb[h][:], qT[h])

            for h in range(n_heads):
                g = h // heads_per_kv
                for j in range(sq_blocks):
                    jsl = bass.ts(j, sq_blk)
                    o_ps = opsum.tile([128, sq_blk], F32, tag="o", name="o_ps")
                    l_ps = lpsum.tile([1, sq_blk], F32, tag="l", name="l_ps")
                    for t2 in range(sk_tiles // pair):
                        s_ps = spsum.tile([128, pair * sq_blk], F32, tag="s", name="s_ps")
                        p_sb = ppool.tile([128, pair * sq_blk], F32R, tag="p", name="p_sb")
                        for u in range(pair):
                            t = t2 * pair + u
                            nc.tensor.matmul(
                                s_ps[:, bass.ts(u, sq_blk)],
                                k_sb[g][:, bass.ts(t, 128)],
                                q_sb[h][:, jsl],
                                start=True,
                                stop=True,
                            )
                        nc.scalar.activation(p_sb[:], s_ps[:], EXP, scale=SCALE)
                        for u in range(pair):
                            t = t2 * pair + u
                            ph = p_sb[:, bass.ts(u, sq_blk)]
                            nc.tensor.matmul(
                                l_ps[:],
                                ones_sb[:],
                                ph,
                                start=(t == 0),
                                stop=(t == sk_tiles - 1),
                                skip_group_check=True,
                            )
                            nc.tensor.matmul(
                                o_ps[:],
                                v_sb[g][:, bass.ts(t, 128)].bitcast(F32R),
                                ph,
                                start=(t == 0),
                                stop=(t == sk_tiles - 1),
                                skip_group_check=True,
                            )
                    rl_sb = rpool.tile([1, sq_blk], F32, tag="rl", name="rl_sb")
                    nc.vector.reciprocal(rl_sb[:], l_ps[:])
                    bc_sb = bpool.tile([128, sq_blk], F32, tag="bc", name="bc_sb")
                    nc.gpsimd.partition_broadcast(bc_sb[:], rl_sb[:])
                    ot_sb = outp.tile([128, sq_blk], F32, tag="ot", name="ot_sb")
                    nc.vector.tensor_mul(ot_sb[:], o_ps[:], bc_sb[:])
                    nc.sync.dma_start(oT[h, :, jsl], ot_sb[:])

    nc.compile()
    return nc


_NC_CACHE = {}


def _get_nc():
    if "nc" not in _NC_CACHE:
        _NC_CACHE["nc"] = build_nc()
    return _NC_CACHE["nc"]


def make_in_maps(q, kv):
    q = np.asarray(q)
    kv = np.asarray(kv)
    k = kv[:, :, 0]  # [B, Sk, Hkv, D]
    v = kv[:, :, 1]  # [B, Sk, Hkv, D]
    # head-major transposed layouts
    qT_all = np.ascontiguousarray(q.transpose(0, 2, 3, 1))  # [B, H, D, Sq]
    kT_all = np.ascontiguousarray(k.transpose(0, 2, 3, 1))  # [B, Hkv, D, Sk]
    # vt[b, hkv, p, t, d] = v[b, t*128 + p, hkv, d]
    vt_all = np.ascontiguousarray(
        v.reshape(B, SK // 128, 128, HKV, D).transpose(0, 3, 2, 1, 4)
    ).reshape(B, HKV, 128, (SK // 128) * D)
    ones = np.ones((128, 1), np.float32)

    in_maps = []
    for c in range(N_CORES):
        b = c // (N_CORES // B)
        part = c % (N_CORES // B)
        h0 = part * H_PER_CORE
        g0 = part * KV_PER_CORE
        in_maps.append(
            {
                "qT": qT_all[b, h0 : h0 + H_PER_CORE],
                "kT": kT_all[b, g0 : g0 + KV_PER_CORE],
                "vt": vt_all[b, g0 : g0 + KV_PER_CORE],
                "ones": ones,
            }
        )
    return in_maps


def gather_output(results):
    out = np.empty((B, SQ, H, D), np.float32)
    for c in range(N_CORES):
        b = c // (N_CORES // B)
        part = c % (N_CORES // B)
        h0 = part * H_PER_CORE
        # oT [n_heads, D, Sq] -> [Sq, n_heads, D]
        out[b, :, h0 : h0 + H_PER_CORE, :] = results[c]["oT"].transpose(2, 0, 1)
    return out


def run(q, kv, trace=False, **kwargs):
    nc = _get_nc()
    in_maps = make_in_maps(q, kv)
    res = run_bass_kernel_spmd(
        nc, in_maps, core_ids=list(range(N_CORES)), trace=trace, **kwargs
    )
    return gather_output(res.results), res


def kernel(q, kv):
    out, _ = run(q, kv, trace=False)
    return out


# revision 4
# speedup vs baseline: 1.2399x; 1.2399x over previous
"""GQA cross-attention kernel for 8 trn2 NeuronCores.

Problem: q [2, 2048, 32, 128] fp32, kv [2, 2048, 2, 8, 128] fp32
         -> softmax(q @ k^T / sqrt(128)) @ v  -> [2, 2048, 32, 128]

Sharding: 64 (batch, head) units over 8 cores: core c gets batch c//4,
q-heads [8*(c%4), 8*(c%4)+8) and kv-heads [2*(c%4), 2*(c%4)+2).

Device layout (host pre-transposes, free):
  qT  [8, 128, 2048]  = q head-major, D on partitions (fp32r)
  kT  [2, 128, 2048]  = k head-major, D on partitions (fp32r)
  vt  [2, 128, 2048]  = v tiled: vt[i, p, t*128+d] = v[t*128+p, d] (bf16)
  oT  [8, 128, 2048]  = output O^T per head (host transposes back)

Per (head, 512-wide q block): stream 16 k-tiles of 128:
  S^T tile = K_tile^T . Q_block   (fp32r matmul, [128 sk, 512 sq] PSUM)
  P = exp(scale * S^T)            (ScalarE, PSUM->SBUF bf16; scores ~N(0,1)
                                   so no max subtraction needed)
  O^T += V_tile^T . P             (bf16 matmul, PSUM accumulation)
then, at block end, row sums l = ones^T . P via 4x column-tiled (128x32
mode) bf16 matmuls packed at PSUM partitions 0/32/64/96, combined on DVE;
epilogue: recip_approx -> partition broadcast -> multiply+evacuate -> DMA.
"""

import math

import numpy as np

import concourse.bass as bass
import concourse.mybir as mybir
import concourse.tile as tile
from concourse import bacc
from concourse.bass_utils import run_bass_kernel_spmd

F32 = mybir.dt.float32
F32R = mybir.dt.float32r
BF16 = mybir.dt.bfloat16
EXP = mybir.ActivationFunctionType.Exp

B, SQ, SK, H, HKV, D = 2, 2048, 2048, 32, 8, 128
N_CORES = 8
H_PER_CORE = H * B // N_CORES  # 8
KV_PER_CORE = HKV * B // N_CORES  # 2
SCALE = 1.0 / math.sqrt(D)


def build_nc(
    n_heads=H_PER_CORE,
    n_kv=KV_PER_CORE,
    sq=SQ,
    sk=SK,
    sq_blk=512,
    pair=2,
    packed_sums=True,
):
    """Build the SPMD Bass program (identical on all cores)."""
    assert n_heads % n_kv == 0
    heads_per_kv = n_heads // n_kv
    sk_tiles = sk // 128
    sq_blocks = sq // sq_blk
    assert sk_tiles % pair == 0
    n_pairs = sk_tiles // pair

    nc = bacc.Bacc("TRN2", target_bir_lowering=False, debug=False)

    qT = nc.dram_tensor("qT", [n_heads, D, sq], F32R, kind="ExternalInput")
    kT = nc.dram_tensor("kT", [n_kv, D, sk], F32R, kind="ExternalInput")
    vt = nc.dram_tensor("vt", [n_kv, 128, sk_tiles * D], BF16, kind="ExternalInput")
    ones = nc.dram_tensor("ones", [128, 1], BF16, kind="ExternalInput")
    oT = nc.dram_tensor("oT", [n_heads, D, sq], F32, kind="ExternalOutput")

    with tile.TileContext(nc) as tc:
        with (
            tc.tile_pool(name="inp", bufs=1) as inp_pool,
            tc.tile_pool(name="ppool", bufs=n_pairs + 2) as ppool,
            tc.tile_pool(name="rpool", bufs=2) as rpool,
            tc.tile_pool(name="bpool", bufs=2) as bpool,
            tc.tile_pool(name="outp", bufs=3) as outp,
            tc.tile_pool(name="spsum", bufs=2, space="PSUM") as spsum,
            tc.tile_pool(name="opsum", bufs=2, space="PSUM") as opsum,
            tc.tile_pool(name="lpsum", bufs=2, space="PSUM") as lpsum,
        ):
            ones_sb = inp_pool.tile([128, 1], BF16, tag="ones", name="ones_sb")
            nc.sync.dma_start(ones_sb[:], ones[:])

            q_sb = [None] * n_heads
            k_sb = [None] * n_kv
            v_sb = [None] * n_kv
            # DMA order: kv group 0 + its q heads first so compute starts early.
            for g in range(n_kv):
                k_sb[g] = inp_pool.tile([D, sk], F32R, tag=f"k{g}", name=f"k_sb{g}")
                v_sb[g] = inp_pool.tile(
                    [128, sk_tiles * D], BF16, tag=f"v{g}", name=f"v_sb{g}"
                )
                nc.sync.dma_start(k_sb[g][:], kT[g])
                nc.sync.dma_start(v_sb[g][:], vt[g])
                for hh in range(heads_per_kv):
                    h = g * heads_per_kv + hh
                    q_sb[h] = inp_pool.tile([D, sq], F32R, tag=f"q{h}", name=f"q_sb{h}")
                    nc.sync.dma_start(q_sb[h][:], qT[h])

            for h in range(n_heads):
                g = h // heads_per_kv
                for j in range(sq_blocks):
                    jsl = bass.ts(j, sq_blk)
                    o_ps = opsum.tile([128, sq_blk], F32, tag="o", name="o_ps")
                    l_ps = lpsum.tile([128, sq_blk], F32, tag="l", name="l_ps")
                    p_tiles = []
                    for t2 in range(n_pairs):
                        s_ps = spsum.tile(
                            [128, pair * sq_blk], F32, tag="s", name="s_ps"
                        )
                        p_sb = ppool.tile(
                            [128, pair * sq_blk], BF16, tag="p", name="p_sb"
                        )
                        for u in range(pair):
                            t = t2 * pair + u
                            nc.tensor.matmul(
                                s_ps[:, bass.ts(u, sq_blk)],
                                k_sb[g][:, bass.ts(t, 128)],
                                q_sb[h][:, jsl],
                                start=True,
                                stop=True,
                            )
                        nc.scalar.activation(p_sb[:], s_ps[:], EXP, scale=SCALE)
                        p_tiles.append(p_sb)
                        for u in range(pair):
                            t = t2 * pair + u
                            nc.tensor.matmul(
                                o_ps[:],
                                v_sb[g][:, bass.ts(t, 128)],
                                p_sb[:, bass.ts(u, sq_blk)],
                                start=(t == 0),
                                stop=(t == sk_tiles - 1),
                                skip_group_check=True,
                            )
                    # row sums: 4x column-tiled (128x32 mode) packed matmuls,
                    # partials land at PSUM partitions 0/32/64/96.
                    if packed_sums:
                        n_pos = 4
                        for t in range(sk_tiles):
                            pos = 32 * (t % n_pos)
                            grp = t // n_pos
                            ph = p_tiles[t // pair][:, bass.ts(t % pair, sq_blk)]
                            nc.tensor.matmul(
                                l_ps[pos : pos + 1, :],
                                ones_sb[:],
                                ph,
                                start=(grp == 0),
                                stop=(grp == sk_tiles // n_pos - 1),
                                tile_position=(0, pos),
                                skip_group_check=True,
                            )
                    else:
                        for t in range(sk_tiles):
                            ph = p_tiles[t // pair][:, bass.ts(t % pair, sq_blk)]
                            nc.tensor.matmul(
                                l_ps[0:1, :],
                                ones_sb[:],
                                ph,
                                start=(t == 0),
                                stop=(t == sk_tiles - 1),
                                skip_group_check=True,
                            )
                    # combine partials + reciprocal (DVE), broadcast (gpsimd)
                    rl_sb = rpool.tile([1, sq_blk], F32, tag="rl", name="rl_sb")
                    if packed_sums:
                        c1 = rpool.tile([1, sq_blk], F32, tag="c1", name="c1")
                        c2 = rpool.tile([1, sq_blk], F32, tag="c2", name="c2")
                        a1 = rpool.tile([1, sq_blk], F32, tag="a1", name="a1")
                        a2 = rpool.tile([1, sq_blk], F32, tag="a2", name="a2")
                        t1 = rpool.tile([1, sq_blk], F32, tag="t1", name="t1")
                        nc.vector.tensor_copy(c1[:], l_ps[32:33, :])
                        nc.vector.tensor_copy(c2[:], l_ps[96:97, :])
                        nc.vector.tensor_add(a1[:], l_ps[0:1, :], c1[:])
                        nc.vector.tensor_add(a2[:], l_ps[64:65, :], c2[:])
                        nc.vector.tensor_add(t1[:], a1[:], a2[:])
                        nc.vector.reciprocal_approx_fast(rl_sb[:], t1[:])
                    else:
                        t1 = rpool.tile([1, sq_blk], F32, tag="t1", name="t1")
                        nc.vector.tensor_copy(t1[:], l_ps[0:1, :])
                        nc.vector.reciprocal_approx_fast(rl_sb[:], t1[:])
                    bc_sb = bpool.tile([128, sq_blk], F32, tag="bc", name="bc_sb")
                    nc.gpsimd.partition_broadcast(bc_sb[:], rl_sb[:])
                    ot_sb = outp.tile([128, sq_blk], F32, tag="ot", name="ot_sb")
                    nc.vector.tensor_mul(ot_sb[:], o_ps[:], bc_sb[:])
                    nc.sync.dma_start(oT[h, :, jsl], ot_sb[:])

    nc.compile()
    return nc


_NC_CACHE = {}


def _get_nc():
    if "nc" not in _NC_CACHE:
        _NC_CACHE["nc"] = build_nc()
    return _NC_CACHE["nc"]


def make_in_maps(q, kv):
    import ml_dtypes

    q = np.asarray(q)
    kv = np.asarray(kv)
    k = kv[:, :, 0]  # [B, Sk, Hkv, D]
    v = kv[:, :, 1]  # [B, Sk, Hkv, D]
    # head-major transposed layouts
    qT_all = np.ascontiguousarray(q.transpose(0, 2, 3, 1))  # [B, H, D, Sq]
    kT_all = np.ascontiguousarray(k.transpose(0, 2, 3, 1))  # [B, Hkv, D, Sk]
    # vt[b, hkv, p, t, d] = v[b, t*128 + p, hkv, d]
    vt_all = np.ascontiguousarray(
        v.reshape(B, SK // 128, 128, HKV, D)
        .transpose(0, 3, 2, 1, 4)
        .astype(ml_dtypes.bfloat16)
    ).reshape(B, HKV, 128, (SK // 128) * D)
    ones = np.ones((128, 1), ml_dtypes.bfloat16)

    in_maps = []
    for c in range(N_CORES):
        b = c // (N_CORES // B)
        part = c % (N_CORES // B)
        h0 = part * H_PER_CORE
        g0 = part * KV_PER_CORE
        in_maps.append(
            {
                "qT": qT_all[b, h0 : h0 + H_PER_CORE],
                "kT": kT_all[b, g0 : g0 + KV_PER_CORE],
                "vt": vt_all[b, g0 : g0 + KV_PER_CORE],
                "ones": ones,
            }
        )
    return in_maps


def gather_output(results):
    out = np.empty((B, SQ, H, D), np.float32)
    for c in range(N_CORES):
        b = c // (N_CORES // B)
        part = c % (N_CORES // B)
        h0 = part * H_PER_CORE
        # oT [n_heads, D, Sq] -> [Sq, n_heads, D]
        out[b, :, h0 : h0 + H_PER_CORE, :] = results[c]["oT"].transpose(2, 0, 1)
    return out


def run(q, kv, trace=False, **kwargs):
    nc = _get_nc()
    in_maps = make_in_maps(q, kv)
    res = run_bass_kernel_spmd(
        nc, in_maps, core_ids=list(range(N_CORES)), trace=trace, **kwargs
    )
    return gather_output(res.results), res


def kernel(q, kv):
    out, _ = run(q, kv, trace=False)
    return out


# revision 7
# speedup vs baseline: 1.2925x; 1.0424x over previous
"""GQA cross-attention kernel for 8 trn2 NeuronCores.

Problem: q [2, 2048, 32, 128] fp32, kv [2, 2048, 2, 8, 128] fp32
         -> softmax(q @ k^T / sqrt(128)) @ v  -> [2, 2048, 32, 128]

Sharding: 64 (batch, head) units over 8 cores: core c gets batch c//4,
q-heads [8*(c%4), 8*(c%4)+8) and kv-heads [2*(c%4), 2*(c%4)+2).

Device layout (host pre-transposes, free):
  qT  [8, 128, 2048]  = q head-major, D on partitions (fp32r)
  kT  [2, 128, 2048]  = k head-major, D on partitions (fp32r)
  vt  [2, 128, 2048]  = v tiled: vt[i, p, t*128+d] = v[t*128+p, d] (bf16)
  oT  [8, 128, 2048]  = output O^T per head (host transposes back)

Per (head, 512-wide q block): stream 16 k-tiles of 128:
  S^T tile = K_tile^T . Q_block   (fp32r matmul, [128 sk, 512 sq] PSUM)
  P = exp(scale * S^T)            (ScalarE, PSUM->SBUF bf16; scores ~N(0,1)
                                   so no max subtraction needed)
  O^T += V_tile^T . P             (bf16 matmul, PSUM accumulation)
then, at block end, row sums l = ones^T . P via 4x column-tiled (128x32
mode) bf16 matmuls packed at PSUM partitions 0/32/64/96, combined on DVE;
epilogue: recip_approx -> partition broadcast -> multiply+evacuate -> DMA.
"""

import math

import numpy as np

import concourse.bass as bass
import concourse.mybir as mybir
import concourse.tile as tile
from concourse import bacc
from concourse.bass import _add_dep_helper
from concourse.bass_utils import run_bass_kernel_spmd

F32 = mybir.dt.float32
F32R = mybir.dt.float32r
BF16 = mybir.dt.bfloat16
EXP = mybir.ActivationFunctionType.Exp

B, SQ, SK, H, HKV, D = 2, 2048, 2048, 32, 8, 128
N_CORES = 8
H_PER_CORE = H * B // N_CORES  # 8
KV_PER_CORE = HKV * B // N_CORES  # 2
SCALE = 1.0 / math.sqrt(D)


def build_nc(
    n_heads=H_PER_CORE,
    n_kv=KV_PER_CORE,
    sq=SQ,
    sk=SK,
    sq_blk=512,
    pair=2,
    packed_sums=True,
):
    """Build the SPMD Bass program (identical on all cores)."""
    assert n_heads % n_kv == 0
    heads_per_kv = n_heads // n_kv
    sk_tiles = sk // 128
    sq_blocks = sq // sq_blk
    assert sk_tiles % pair == 0
    n_pairs = sk_tiles // pair

    nc = bacc.Bacc("TRN2", target_bir_lowering=False, debug=False)

    qT = nc.dram_tensor("qT", [n_heads, D, sq], F32R, kind="ExternalInput")
    kT = nc.dram_tensor("kT", [n_kv, D, sk], F32R, kind="ExternalInput")
    vt = nc.dram_tensor("vt", [n_kv, 128, sk_tiles * D], BF16, kind="ExternalInput")
    ones = nc.dram_tensor("ones", [128, 1], BF16, kind="ExternalInput")
    oT = nc.dram_tensor("oT", [n_heads, D, sq], F32, kind="ExternalOutput")

    with tile.TileContext(nc) as tc:
        with (
            tc.tile_pool(name="inp", bufs=1) as inp_pool,
            tc.tile_pool(name="ppool", bufs=n_pairs + 2) as ppool,
            tc.tile_pool(name="rpool", bufs=2) as rpool,
            tc.tile_pool(name="bpool", bufs=2) as bpool,
            tc.tile_pool(name="outp", bufs=3) as outp,
            tc.tile_pool(name="spsum", bufs=2, space="PSUM") as spsum,
            tc.tile_pool(name="opsum", bufs=2, space="PSUM") as opsum,
            tc.tile_pool(name="lpsum", bufs=2, space="PSUM") as lpsum,
        ):
            ones_sb = inp_pool.tile([128, 1], BF16, tag="ones", name="ones_sb")
            nc.sync.dma_start(ones_sb[:], ones[:])

            q_sb = [None] * n_heads
            k_sb = [None] * n_kv
            v_sb = [None] * n_kv
            # DMA order: kv group 0 + its q heads first so compute starts
            # early; chunk along the free dim so first tiles land fast.
            def chunked_dma(dst, src, n_chunks):
                csz = dst.shape[-1] // n_chunks
                for i in range(n_chunks):
                    nc.sync.dma_start(
                        dst[:, bass.ts(i, csz)], src[:, bass.ts(i, csz)]
                    )

            for g in range(n_kv):
                k_sb[g] = inp_pool.tile([D, sk], F32R, tag=f"k{g}", name=f"k_sb{g}")
                v_sb[g] = inp_pool.tile(
                    [128, sk_tiles * D], BF16, tag=f"v{g}", name=f"v_sb{g}"
                )
                chunked_dma(k_sb[g], kT[g], 4)
                chunked_dma(v_sb[g], vt[g], 4)
                for hh in range(heads_per_kv):
                    h = g * heads_per_kv + hh
                    q_sb[h] = inp_pool.tile([D, sq], F32R, tag=f"q{h}", name=f"q_sb{h}")
                    chunked_dma(q_sb[h], qT[h], 4)

            prev_last_sum = None
            for h in range(n_heads):
                g = h // heads_per_kv
                for j in range(sq_blocks):
                    jsl = bass.ts(j, sq_blk)
                    o_ps = opsum.tile([128, sq_blk], F32, tag="o", name="o_ps")
                    l_ps = lpsum.tile([128, sq_blk], F32, tag="l", name="l_ps")
                    p_tiles = []
                    last_mm = None
                    for t2 in range(n_pairs):
                        s_ps = spsum.tile(
                            [128, pair * sq_blk], F32, tag="s", name="s_ps"
                        )
                        p_sb = ppool.tile(
                            [128, pair * sq_blk], BF16, tag="p", name="p_sb"
                        )
                        for u in range(pair):
                            t = t2 * pair + u
                            mm = nc.tensor.matmul(
                                s_ps[:, bass.ts(u, sq_blk)],
                                k_sb[g][:, bass.ts(t, 128)],
                                q_sb[h][:, jsl],
                                start=True,
                                stop=True,
                            )
                            if t2 == 0 and u == 0 and prev_last_sum is not None:
                                # keep PE in-order across the tiling-mode
                                # switch: block MMs after previous block's sums
                                _add_dep_helper(
                                    mm.ins,
                                    prev_last_sum.ins,
                                    sync=False,
                                    reason="order big MMs after prev sums",
                                )
                        nc.scalar.activation(p_sb[:], s_ps[:], EXP, scale=SCALE)
                        p_tiles.append(p_sb)
                        for u in range(pair):
                            t = t2 * pair + u
                            last_mm = nc.tensor.matmul(
                                o_ps[:],
                                v_sb[g][:, bass.ts(t, 128)],
                                p_sb[:, bass.ts(u, sq_blk)],
                                start=(t == 0),
                                stop=(t == sk_tiles - 1),
                                skip_group_check=True,
                            )
                    # row sums: 4x column-tiled (128x32 mode) packed matmuls,
                    # partials land at PSUM partitions 0/32/64/96. Ordered
                    # contiguously at block end to avoid mode-switch thrash.
                    if packed_sums:
                        n_pos = 4
                        for t in range(sk_tiles):
                            pos = 32 * (t % n_pos)
                            grp = t // n_pos
                            ph = p_tiles[t // pair][:, bass.ts(t % pair, sq_blk)]
                            smm = nc.tensor.matmul(
                                l_ps[pos : pos + 1, :],
                                ones_sb[:],
                                ph,
                                start=(grp == 0),
                                stop=(grp == sk_tiles // n_pos - 1),
                                tile_position=(0, pos),
                                skip_group_check=True,
                            )
                            if t == 0:
                                _add_dep_helper(
                                    smm.ins,
                                    last_mm.ins,
                                    sync=False,
                                    reason="order sums after block MMs",
                                )
                            prev_last_sum = smm
                    else:
                        for t in range(sk_tiles):
                            ph = p_tiles[t // pair][:, bass.ts(t % pair, sq_blk)]
                            nc.tensor.matmul(
                                l_ps[0:1, :],
                                ones_sb[:],
                                ph,
                                start=(t == 0),
                                stop=(t == sk_tiles - 1),
                                skip_group_check=True,
                            )
                    # combine partials + reciprocal (DVE), broadcast (gpsimd)
                    rl_sb = rpool.tile([1, sq_blk], F32, tag="rl", name="rl_sb")
                    if packed_sums:
                        c1 = rpool.tile([1, sq_blk], F32, tag="c1", name="c1")
                        c2 = rpool.tile([1, sq_blk], F32, tag="c2", name="c2")
                        a1 = rpool.tile([1, sq_blk], F32, tag="a1", name="a1")
                        a2 = rpool.tile([1, sq_blk], F32, tag="a2", name="a2")
                        t1 = rpool.tile([1, sq_blk], F32, tag="t1", name="t1")
                        nc.vector.tensor_copy(c1[:], l_ps[32:33, :])
                        nc.vector.tensor_copy(c2[:], l_ps[96:97, :])
                        nc.vector.tensor_add(a1[:], l_ps[0:1, :], c1[:])
                        nc.vector.tensor_add(a2[:], l_ps[64:65, :], c2[:])
                        nc.vector.tensor_add(t1[:], a1[:], a2[:])
                        nc.vector.reciprocal_approx_fast(rl_sb[:], t1[:])
                    else:
                        t1 = rpool.tile([1, sq_blk], F32, tag="t1", name="t1")
                        nc.vector.tensor_copy(t1[:], l_ps[0:1, :])
                        nc.vector.reciprocal_approx_fast(rl_sb[:], t1[:])
                    bc_sb = bpool.tile([128, sq_blk], F32, tag="bc", name="bc_sb")
                    nc.gpsimd.partition_broadcast(bc_sb[:], rl_sb[:])
                    ot_sb = outp.tile([128, sq_blk], F32, tag="ot", name="ot_sb")
                    nc.vector.tensor_mul(ot_sb[:], o_ps[:], bc_sb[:])
                    nc.sync.dma_start(oT[h, :, jsl], ot_sb[:])

    nc.compile()
    return nc


_NC_CACHE = {}


def _get_nc():
    if "nc" not in _NC_CACHE:
        _NC_CACHE["nc"] = build_nc()
    return _NC_CACHE["nc"]


def make_in_maps(q, kv):
    import ml_dtypes

    q = np.asarray(q)
    kv = np.asarray(kv)
    k = kv[:, :, 0]  # [B, Sk, Hkv, D]
    v = kv[:, :, 1]  # [B, Sk, Hkv, D]
    # head-major transposed layouts
    qT_all = np.ascontiguousarray(q.transpose(0, 2, 3, 1))  # [B, H, D, Sq]
    kT_all = np.ascontiguousarray(k.transpose(0, 2, 3, 1))  # [B, Hkv, D, Sk]
    # vt[b, hkv, p, t, d] = v[b, t*128 + p, hkv, d]
    vt_all = np.ascontiguousarray(
        v.reshape(B, SK // 128, 128, HKV, D)
        .transpose(0, 3, 2, 1, 4)
        .astype(ml_dtypes.bfloat16)
    ).reshape(B, HKV, 128, (SK // 128) * D)
    ones = np.ones((128, 1), ml_dtypes.bfloat16)

    in_maps = []
    for c in range(N_CORES):
        b = c // (N_CORES // B)
        part = c % (N_CORES // B)
        h0 = part * H_PER_CORE
        g0 = part * KV_PER_CORE
        in_maps.append(
            {
                "qT": qT_all[b, h0 : h0 + H_PER_CORE],
                "kT": kT_all[b, g0 : g0 + KV_PER_CORE],
                "vt": vt_all[b, g0 : g0 + KV_PER_CORE],
                "ones": ones,
            }
        )
    return in_maps


def gather_output(results):
    out = np.empty((B, SQ, H, D), np.float32)
    for c in range(N_CORES):
        b = c // (N_CORES // B)
        part = c % (N_CORES // B)
        h0 = part * H_PER_CORE
        # oT [n_heads, D, Sq] -> [Sq, n_heads, D]
        out[b, :, h0 : h0 + H_PER_CORE, :] = results[c]["oT"].transpose(2, 0, 1)
    return out


def run(q, kv, trace=False, **kwargs):
    nc = _get_nc()
    in_maps = make_in_maps(q, kv)
    res = run_bass_kernel_spmd(
        nc, in_maps, core_ids=list(range(N_CORES)), trace=trace, **kwargs
    )
    return gather_output(res.results), res


def kernel(q, kv):
    out, _ = run(q, kv, trace=False)
    return out


# revision 8
# speedup vs baseline: 1.4005x; 1.0835x over previous
"""GQA cross-attention kernel for 8 trn2 NeuronCores.

Problem: q [2, 2048, 32, 128] fp32, kv [2, 2048, 2, 8, 128] fp32
         -> softmax(q @ k^T / sqrt(128)) @ v  -> [2, 2048, 32, 128]

Sharding: 64 (batch, head) units over 8 cores: core c gets batch c//4,
q-heads [8*(c%4), 8*(c%4)+8) and kv-heads [2*(c%4), 2*(c%4)+2).

Device layout (host pre-transposes, free):
  qT  [8, 128, 2048]  = q head-major, D on partitions (fp32r)
  kT  [2, 128, 2048]  = k head-major, D on partitions (fp32r)
  vt  [2, 128, 2048]  = v tiled: vt[i, p, t*128+d] = v[t*128+p, d] (bf16)
  oT  [8, 128, 2048]  = output O^T per head (host transposes back)

Per (head, 512-wide q block): stream 16 k-tiles of 128:
  S^T tile = K_tile^T . Q_block   (fp32r matmul, [128 sk, 512 sq] PSUM)
  P = exp(scale * S^T)            (ScalarE, PSUM->SBUF bf16; scores ~N(0,1)
                                   so no max subtraction needed)
  O^T += V_tile^T . P             (bf16 matmul, PSUM accumulation)
then, at block end, row sums l = ones^T . P via 4x column-tiled (128x32
mode) bf16 matmuls packed at PSUM partitions 0/32/64/96, combined on DVE;
epilogue: recip_approx -> partition broadcast -> multiply+evacuate -> DMA.
"""

import math

import numpy as np

import concourse.bass as bass
import concourse.mybir as mybir
import concourse.tile as tile
from concourse import bacc
from concourse.bass import _add_dep_helper
from concourse.bass_utils import run_bass_kernel_spmd

F32 = mybir.dt.float32
F32R = mybir.dt.float32r
BF16 = mybir.dt.bfloat16
EXP = mybir.ActivationFunctionType.Exp

B, SQ, SK, H, HKV, D = 2, 2048, 2048, 32, 8, 128
N_CORES = 8
H_PER_CORE = H * B // N_CORES  # 8
KV_PER_CORE = HKV * B // N_CORES  # 2
SCALE = 1.0 / math.sqrt(D)


def build_nc(
    n_heads=H_PER_CORE,
    n_kv=KV_PER_CORE,
    sq=SQ,
    sk=SK,
    sq_blk=512,
    pair=2,
    packed_sums=True,
):
    """Build the SPMD Bass program (identical on all cores)."""
    assert n_heads % n_kv == 0
    heads_per_kv = n_heads // n_kv
    sk_tiles = sk // 128
    sq_blocks = sq // sq_blk
    assert sk_tiles % pair == 0
    n_pairs = sk_tiles // pair

    nc = bacc.Bacc("TRN2", target_bir_lowering=False, debug=False)

    qT = nc.dram_tensor("qT", [n_heads, D, sq], F32R, kind="ExternalInput")
    kT = nc.dram_tensor("kT", [n_kv, D, sk], F32R, kind="ExternalInput")
    vt = nc.dram_tensor("vt", [n_kv, 128, sk_tiles * D], BF16, kind="ExternalInput")
    ones = nc.dram_tensor("ones", [128, 1], BF16, kind="ExternalInput")
    oT = nc.dram_tensor("oT", [n_heads, D, sq], F32, kind="ExternalOutput")

    with tile.TileContext(nc) as tc:
        with (
            tc.tile_pool(name="inp", bufs=1) as inp_pool,
            tc.tile_pool(name="ppool", bufs=n_pairs + 2) as ppool,
            tc.tile_pool(name="rpool", bufs=2) as rpool,
            tc.tile_pool(name="bpool", bufs=2) as bpool,
            tc.tile_pool(name="outp", bufs=3) as outp,
            tc.tile_pool(name="spsum", bufs=2, space="PSUM") as spsum,
            tc.tile_pool(name="opsum", bufs=2, space="PSUM") as opsum,
            tc.tile_pool(name="lpsum", bufs=2, space="PSUM") as lpsum,
        ):
            ones_sb = inp_pool.tile([128, 1], BF16, tag="ones", name="ones_sb")
            nc.sync.dma_start(ones_sb[:], ones[:])

            q_sb = [None] * n_heads
            k_sb = [None] * n_kv
            v_sb = [None] * n_kv
            # DMA order: kv group 0 + its q heads first so compute starts
            # early; chunk along the free dim so first tiles land fast.
            def chunked_dma(dst, src, n_chunks):
                csz = dst.shape[-1] // n_chunks
                for i in range(n_chunks):
                    nc.sync.dma_start(
                        dst[:, bass.ts(i, csz)], src[:, bass.ts(i, csz)]
                    )

            for g in range(n_kv):
                k_sb[g] = inp_pool.tile([D, sk], F32R, tag=f"k{g}", name=f"k_sb{g}")
                v_sb[g] = inp_pool.tile(
                    [128, sk_tiles * D], BF16, tag=f"v{g}", name=f"v_sb{g}"
                )
                chunked_dma(k_sb[g], kT[g], 4)
                chunked_dma(v_sb[g], vt[g], 4)
                for hh in range(heads_per_kv):
                    h = g * heads_per_kv + hh
                    q_sb[h] = inp_pool.tile([D, sq], F32R, tag=f"q{h}", name=f"q_sb{h}")
                    chunked_dma(q_sb[h], qT[h], 4)

            # Software-pipelined emission, one pair of lookahead: MM1+exp for
            # step P are emitted before MM2 of step P-1, so the PE always has
            # next-step MM1 work during the previous exp's latency — including
            # across block boundaries (where the sum burst + tiling-mode
            # switch would otherwise stall both PE and ScalarE).
            blocks = [(h, j) for h in range(n_heads) for j in range(sq_blocks)]
            n_blocks = len(blocks)
            state = {}  # per-block: o_ps, l_ps, p_tiles
            prev = None  # (block_idx, t2, p_sb)
            prev_last_sum = None

            def emit_mm2(bi, t2, p_sb):
                h, j = blocks[bi]
                g = h // heads_per_kv
                st = state[bi]
                for u in range(pair):
                    t = t2 * pair + u
                    st["last_mm"] = nc.tensor.matmul(
                        st["o_ps"][:],
                        v_sb[g][:, bass.ts(t, 128)],
                        p_sb[:, bass.ts(u, sq_blk)],
                        start=(t == 0),
                        stop=(t == sk_tiles - 1),
                        skip_group_check=True,
                    )

            def emit_block_tail(bi):
                nonlocal prev_last_sum
                h, j = blocks[bi]
                jsl = bass.ts(j, sq_blk)
                st = state.pop(bi)
                l_ps, o_ps, p_tiles = st["l_ps"], st["o_ps"], st["p_tiles"]
                # row sums: 4x column-tiled (128x32 mode) packed matmuls,
                # partials at PSUM partitions 0/32/64/96; kept contiguous.
                n_pos = 4
                for t in range(sk_tiles):
                    pos = 32 * (t % n_pos)
                    grp = t // n_pos
                    ph = p_tiles[t // pair][:, bass.ts(t % pair, sq_blk)]
                    smm = nc.tensor.matmul(
                        l_ps[pos : pos + 1, :],
                        ones_sb[:],
                        ph,
                        start=(grp == 0),
                        stop=(grp == sk_tiles // n_pos - 1),
                        tile_position=(0, pos),
                        skip_group_check=True,
                    )
                    if t == 0:
                        _add_dep_helper(
                            smm.ins,
                            st["last_mm"].ins,
                            sync=False,
                            reason="order sums after block MMs",
                        )
                    prev_last_sum = smm
                # combine partials + reciprocal (DVE), broadcast (gpsimd)
                rl_sb = rpool.tile([1, sq_blk], F32, tag="rl", name="rl_sb")
                c1 = rpool.tile([1, sq_blk], F32, tag="c1", name="c1")
                c2 = rpool.tile([1, sq_blk], F32, tag="c2", name="c2")
                a1 = rpool.tile([1, sq_blk], F32, tag="a1", name="a1")
                a2 = rpool.tile([1, sq_blk], F32, tag="a2", name="a2")
                t1 = rpool.tile([1, sq_blk], F32, tag="t1", name="t1")
                nc.vector.tensor_copy(c1[:], l_ps[32:33, :])
                nc.vector.tensor_copy(c2[:], l_ps[96:97, :])
                nc.vector.tensor_add(a1[:], l_ps[0:1, :], c1[:])
                nc.vector.tensor_add(a2[:], l_ps[64:65, :], c2[:])
                nc.vector.tensor_add(t1[:], a1[:], a2[:])
                nc.vector.reciprocal_approx_fast(rl_sb[:], t1[:])
                bc_sb = bpool.tile([128, sq_blk], F32, tag="bc", name="bc_sb")
                nc.gpsimd.partition_broadcast(bc_sb[:], rl_sb[:])
                ot_sb = outp.tile([128, sq_blk], F32, tag="ot", name="ot_sb")
                nc.vector.tensor_mul(ot_sb[:], o_ps[:], bc_sb[:])
                nc.sync.dma_start(oT[h, :, jsl], ot_sb[:])

            for bi in range(n_blocks):
                h, j = blocks[bi]
                g = h // heads_per_kv
                jsl = bass.ts(j, sq_blk)
                state[bi] = {
                    "o_ps": opsum.tile([128, sq_blk], F32, tag="o", name="o_ps"),
                    "l_ps": lpsum.tile([128, sq_blk], F32, tag="l", name="l_ps"),
                    "p_tiles": [],
                    "last_mm": None,
                }
                for t2 in range(n_pairs):
                    s_ps = spsum.tile([128, pair * sq_blk], F32, tag="s", name="s_ps")
                    p_sb = ppool.tile(
                        [128, pair * sq_blk], BF16, tag="p", name="p_sb"
                    )
                    first_of_block = t2 == 0
                    second_of_block = t2 == 1
                    for u in range(pair):
                        t = t2 * pair + u
                        mm = nc.tensor.matmul(
                            s_ps[:, bass.ts(u, sq_blk)],
                            k_sb[g][:, bass.ts(t, 128)],
                            q_sb[h][:, jsl],
                            start=True,
                            stop=True,
                        )
                        if second_of_block and u == 0 and prev_last_sum is not None:
                            # the lookahead pair (t2==0) may run during the
                            # previous block's exp tail; everything after it
                            # stays ordered behind the previous sum burst.
                            _add_dep_helper(
                                mm.ins,
                                prev_last_sum.ins,
                                sync=False,
                                reason="order big MMs after prev sums",
                            )
                    nc.scalar.activation(p_sb[:], s_ps[:], EXP, scale=SCALE)
                    state[bi]["p_tiles"].append(p_sb)
                    # deferred work from the previous step
                    if prev is not None:
                        pbi, pt2, pp = prev
                        emit_mm2(pbi, pt2, pp)
                        if pt2 == n_pairs - 1:
                            emit_block_tail(pbi)
                    prev = (bi, t2, p_sb)
            # drain the pipeline
            pbi, pt2, pp = prev
            emit_mm2(pbi, pt2, pp)
            emit_block_tail(pbi)

    nc.compile()
    return nc


_NC_CACHE = {}


def _get_nc():
    if "nc" not in _NC_CACHE:
        _NC_CACHE["nc"] = build_nc()
    return _NC_CACHE["nc"]


def make_in_maps(q, kv):
    import ml_dtypes

    q = np.asarray(q)
    kv = np.asarray(kv)
    k = kv[:, :, 0]  # [B, Sk, Hkv, D]
    v = kv[:, :, 1]  # [B, Sk, Hkv, D]
    # head-major transposed layouts
    qT_all = np.ascontiguousarray(q.transpose(0, 2, 3, 1))  # [B, H, D, Sq]
    kT_all = np.ascontiguousarray(k.transpose(0, 2, 3, 1))  # [B, Hkv, D, Sk]
    # vt[b, hkv, p, t, d] = v[b, t*128 + p, hkv, d]
    vt_all = np.ascontiguousarray(
        v.reshape(B, SK // 128, 128, HKV, D)
        .transpose(0, 3, 2, 1, 4)
        .astype(ml_dtypes.bfloat16)
    ).reshape(B, HKV, 128, (SK // 128) * D)
    ones = np.ones((128, 1), ml_dtypes.bfloat16)

    in_maps = []
    for c in range(N_CORES):
        b = c // (N_CORES // B)
        part = c % (N_CORES // B)
        h0 = part * H_PER_CORE
        g0 = part * KV_PER_CORE
        in_maps.append(
            {
                "qT": qT_all[b, h0 : h0 + H_PER_CORE],
                "kT": kT_all[b, g0 : g0 + KV_PER_CORE],
                "vt": vt_all[b, g0 : g0 + KV_PER_CORE],
                "ones": ones,
            }
        )
    return in_maps


def gather_output(results):
    out = np.empty((B, SQ, H, D), np.float32)
    for c in range(N_CORES):
        b = c // (N_CORES // B)
        part = c % (N_CORES // B)
        h0 = part * H_PER_CORE
        # oT [n_heads, D, Sq] -> [Sq, n_heads, D]
        out[b, :, h0 : h0 + H_PER_CORE, :] = results[c]["oT"].transpose(2, 0, 1)
    return out


def run(q, kv, trace=False, **kwargs):
    nc = _get_nc()
    in_maps = make_in_maps(q, kv)
    res = run_bass_kernel_spmd(
        nc, in_maps, core_ids=list(range(N_CORES)), trace=trace, **kwargs
    )
    return gather_output(res.results), res


def kernel(q, kv):
    out, _ = run(q, kv, trace=False)
    return out


# revision 9
# speedup vs baseline: 1.4151x; 1.0105x over previous
"""GQA cross-attention kernel for 8 trn2 NeuronCores.

Problem: q [2, 2048, 32, 128] fp32, kv [2, 2048, 2, 8, 128] fp32
         -> softmax(q @ k^T / sqrt(128)) @ v  -> [2, 2048, 32, 128]

Sharding: 64 (batch, head) units over 8 cores: core c gets batch c//4,
q-heads [8*(c%4), 8*(c%4)+8) and kv-heads [2*(c%4), 2*(c%4)+2).

Device layout (host pre-transposes, free):
  qT  [8, 128, 2048]  = q head-major, D on partitions (fp32r)
  kT  [2, 128, 2048]  = k head-major, D on partitions (fp32r)
  vt  [2, 128, 2048]  = v tiled: vt[i, p, t*128+d] = v[t*128+p, d] (bf16)
  oT  [8, 128, 2048]  = output O^T per head (host transposes back)

Per (head, 512-wide q block): stream 16 k-tiles of 128:
  S^T tile = K_tile^T . Q_block   (fp32r matmul, [128 sk, 512 sq] PSUM)
  P = exp(scale * S^T)            (ScalarE, PSUM->SBUF bf16; scores ~N(0,1)
                                   so no max subtraction needed)
  O^T += V_tile^T . P             (bf16 matmul, PSUM accumulation)
then, at block end, row sums l = ones^T . P via 4x column-tiled (128x32
mode) bf16 matmuls packed at PSUM partitions 0/32/64/96, combined on DVE;
epilogue: recip_approx -> partition broadcast -> multiply+evacuate -> DMA.
"""

import math

import numpy as np

import concourse.bass as bass
import concourse.mybir as mybir
import concourse.tile as tile
from concourse import bacc
from concourse.bass import _add_dep_helper
from concourse.bass_utils import run_bass_kernel_spmd

F32 = mybir.dt.float32
F32R = mybir.dt.float32r
BF16 = mybir.dt.bfloat16
EXP = mybir.ActivationFunctionType.Exp

B, SQ, SK, H, HKV, D = 2, 2048, 2048, 32, 8, 128
N_CORES = 8
H_PER_CORE = H * B // N_CORES  # 8
KV_PER_CORE = HKV * B // N_CORES  # 2
SCALE = 1.0 / math.sqrt(D)


def build_nc(
    n_heads=H_PER_CORE,
    n_kv=KV_PER_CORE,
    sq=SQ,
    sk=SK,
    sq_blk=512,
    pair=2,
    packed_sums=True,
):
    """Build the SPMD Bass program (identical on all cores)."""
    assert n_heads % n_kv == 0
    heads_per_kv = n_heads // n_kv
    sk_tiles = sk // 128
    sq_blocks = sq // sq_blk
    assert sk_tiles % pair == 0
    n_pairs = sk_tiles // pair

    nc = bacc.Bacc("TRN2", target_bir_lowering=False, debug=False)

    qT = nc.dram_tensor("qT", [n_heads, D, sq], F32R, kind="ExternalInput")
    kT = nc.dram_tensor("kT", [n_kv, D, sk], F32R, kind="ExternalInput")
    vt = nc.dram_tensor("vt", [n_kv, 128, sk_tiles * D], BF16, kind="ExternalInput")
    ones = nc.dram_tensor("ones", [128, 1], BF16, kind="ExternalInput")
    oT = nc.dram_tensor("oT", [n_heads, D, sq], F32, kind="ExternalOutput")

    with tile.TileContext(nc) as tc:
        with (
            tc.tile_pool(name="inp", bufs=1) as inp_pool,
            tc.tile_pool(name="ppool", bufs=n_pairs + 2) as ppool,
            tc.tile_pool(name="rpool", bufs=2) as rpool,
            tc.tile_pool(name="bpool", bufs=2) as bpool,
            tc.tile_pool(name="outp", bufs=3) as outp,
            tc.tile_pool(name="spsum", bufs=2, space="PSUM") as spsum,
            tc.tile_pool(name="opsum", bufs=2, space="PSUM") as opsum,
            tc.tile_pool(name="lpsum", bufs=2, space="PSUM") as lpsum,
        ):
            ones_sb = inp_pool.tile([128, 1], BF16, tag="ones", name="ones_sb")
            nc.sync.dma_start(ones_sb[:], ones[:])

            q_sb = [None] * n_heads
            k_sb = [None] * n_kv
            v_sb = [None] * n_kv
            # DMA order: kv group 0 + its q heads first so compute starts
            # early; chunk along the free dim so first tiles land fast.
            def chunked_dma(dst, src, n_chunks):
                csz = dst.shape[-1] // n_chunks
                for i in range(n_chunks):
                    nc.sync.dma_start(
                        dst[:, bass.ts(i, csz)], src[:, bass.ts(i, csz)]
                    )

            for g in range(n_kv):
                k_sb[g] = inp_pool.tile([D, sk], F32R, tag=f"k{g}", name=f"k_sb{g}")
                v_sb[g] = inp_pool.tile(
                    [128, sk_tiles * D], BF16, tag=f"v{g}", name=f"v_sb{g}"
                )
                for hh in range(heads_per_kv):
                    h = g * heads_per_kv + hh
                    q_sb[h] = inp_pool.tile([D, sq], F32R, tag=f"q{h}", name=f"q_sb{h}")
            # first wave: the chunks the first block needs, in need-order, so
            # compute ramps while the rest of the inputs stream in.
            csz = sk // 4
            for i in range(4):
                nc.sync.dma_start(k_sb[0][:, bass.ts(i, csz)], kT[0][:, bass.ts(i, csz)])
                nc.sync.dma_start(q_sb[0][:, bass.ts(i, csz)], qT[0][:, bass.ts(i, csz)])
                nc.sync.dma_start(v_sb[0][:, bass.ts(i, csz)], vt[0][:, bass.ts(i, csz)])
            for h in range(1, heads_per_kv):
                chunked_dma(q_sb[h], qT[h], 4)
            for g in range(1, n_kv):
                chunked_dma(k_sb[g], kT[g], 4)
                chunked_dma(v_sb[g], vt[g], 4)
                for hh in range(heads_per_kv):
                    h = g * heads_per_kv + hh
                    chunked_dma(q_sb[h], qT[h], 4)

            # Software-pipelined emission, one pair of lookahead: MM1+exp for
            # step P are emitted before MM2 of step P-1, so the PE always has
            # next-step MM1 work during the previous exp's latency — including
            # across block boundaries (where the sum burst + tiling-mode
            # switch would otherwise stall both PE and ScalarE).
            blocks = [(h, j) for h in range(n_heads) for j in range(sq_blocks)]
            n_blocks = len(blocks)
            state = {}  # per-block: o_ps, l_ps, p_tiles
            prev = None  # (block_idx, t2, p_sb)
            prev_last_sum = None

            def emit_mm2(bi, t2, p_sb):
                h, j = blocks[bi]
                g = h // heads_per_kv
                st = state[bi]
                for u in range(pair):
                    t = t2 * pair + u
                    st["last_mm"] = nc.tensor.matmul(
                        st["o_ps"][:],
                        v_sb[g][:, bass.ts(t, 128)],
                        p_sb[:, bass.ts(u, sq_blk)],
                        start=(t == 0),
                        stop=(t == sk_tiles - 1),
                        skip_group_check=True,
                    )

            def emit_block_tail(bi):
                nonlocal prev_last_sum
                h, j = blocks[bi]
                jsl = bass.ts(j, sq_blk)
                st = state.pop(bi)
                l_ps, o_ps, p_tiles = st["l_ps"], st["o_ps"], st["p_tiles"]
                # row sums: 4x column-tiled (128x32 mode) packed matmuls,
                # partials at PSUM partitions 0/32/64/96; kept contiguous.
                n_pos = 4
                for t in range(sk_tiles):
                    pos = 32 * (t % n_pos)
                    grp = t // n_pos
                    ph = p_tiles[t // pair][:, bass.ts(t % pair, sq_blk)]
                    smm = nc.tensor.matmul(
                        l_ps[pos : pos + 1, :],
                        ones_sb[:],
                        ph,
                        start=(grp == 0),
                        stop=(grp == sk_tiles // n_pos - 1),
                        tile_position=(0, pos),
                        skip_group_check=True,
                    )
                    if t == 0:
                        _add_dep_helper(
                            smm.ins,
                            st["last_mm"].ins,
                            sync=False,
                            reason="order sums after block MMs",
                        )
                    prev_last_sum = smm
                # combine partials + reciprocal (DVE), broadcast (gpsimd)
                rl_sb = rpool.tile([1, sq_blk], F32, tag="rl", name="rl_sb")
                c1 = rpool.tile([1, sq_blk], F32, tag="c1", name="c1")
                c2 = rpool.tile([1, sq_blk], F32, tag="c2", name="c2")
                a1 = rpool.tile([1, sq_blk], F32, tag="a1", name="a1")
                a2 = rpool.tile([1, sq_blk], F32, tag="a2", name="a2")
                t1 = rpool.tile([1, sq_blk], F32, tag="t1", name="t1")
                nc.vector.tensor_copy(c1[:], l_ps[32:33, :])
                nc.vector.tensor_copy(c2[:], l_ps[96:97, :])
                nc.vector.tensor_add(a1[:], l_ps[0:1, :], c1[:])
                nc.vector.tensor_add(a2[:], l_ps[64:65, :], c2[:])
                nc.vector.tensor_add(t1[:], a1[:], a2[:])
                nc.vector.reciprocal_approx_fast(rl_sb[:], t1[:])
                bc_sb = bpool.tile([128, sq_blk], F32, tag="bc", name="bc_sb")
                nc.gpsimd.partition_broadcast(bc_sb[:], rl_sb[:])
                ot_sb = outp.tile([128, sq_blk], F32, tag="ot", name="ot_sb")
                nc.vector.tensor_mul(ot_sb[:], o_ps[:], bc_sb[:])
                nc.sync.dma_start(oT[h, :, jsl], ot_sb[:])

            for bi in range(n_blocks):
                h, j = blocks[bi]
                g = h // heads_per_kv
                jsl = bass.ts(j, sq_blk)
                state[bi] = {
                    "o_ps": opsum.tile([128, sq_blk], F32, tag="o", name="o_ps"),
                    "l_ps": lpsum.tile([128, sq_blk], F32, tag="l", name="l_ps"),
                    "p_tiles": [],
                    "last_mm": None,
                }
                for t2 in range(n_pairs):
                    s_ps = spsum.tile([128, pair * sq_blk], F32, tag="s", name="s_ps")
                    p_sb = ppool.tile(
                        [128, pair * sq_blk], BF16, tag="p", name="p_sb"
                    )
                    first_of_block = t2 == 0
                    second_of_block = t2 == 1
                    for u in range(pair):
                        t = t2 * pair + u
                        mm = nc.tensor.matmul(
                            s_ps[:, bass.ts(u, sq_blk)],
                            k_sb[g][:, bass.ts(t, 128)],
                            q_sb[h][:, jsl],
                            start=True,
                            stop=True,
                        )
                        if second_of_block and u == 0 and prev_last_sum is not None:
                            # the lookahead pair (t2==0) may run during the
                            # previous block's exp tail; everything after it
                            # stays ordered behind the previous sum burst.
                            _add_dep_helper(
                                mm.ins,
                                prev_last_sum.ins,
                                sync=False,
                                reason="order big MMs after prev sums",
                            )
                    nc.scalar.activation(p_sb[:], s_ps[:], EXP, scale=SCALE)
                    state[bi]["p_tiles"].append(p_sb)
                    # deferred work from the previous step
                    if prev is not None:
                        pbi, pt2, pp = prev
                        emit_mm2(pbi, pt2, pp)
                        if pt2 == n_pairs - 1:
                            emit_block_tail(pbi)
                    prev = (bi, t2, p_sb)
            # drain the pipeline
            pbi, pt2, pp = prev
            emit_mm2(pbi, pt2, pp)
            emit_block_tail(pbi)

    nc.compile()
    return nc


_NC_CACHE = {}


def _get_nc():
    if "nc" not in _NC_CACHE:
        _NC_CACHE["nc"] = build_nc()
    return _NC_CACHE["nc"]


def make_in_maps(q, kv):
    import ml_dtypes

    q = np.asarray(q)
    kv = np.asarray(kv)
    k = kv[:, :, 0]  # [B, Sk, Hkv, D]
    v = kv[:, :, 1]  # [B, Sk, Hkv, D]
    # head-major transposed layouts
    qT_all = np.ascontiguousarray(q.transpose(0, 2, 3, 1))  # [B, H, D, Sq]
    kT_all = np.ascontiguousarray(k.transpose(0, 2, 3, 1))  # [B, Hkv, D, Sk]
    # vt[b, hkv, p, t, d] = v[b, t*128 + p, hkv, d]
    vt_all = np.ascontiguousarray(
        v.reshape(B, SK // 128, 128, HKV, D)
        .transpose(0, 3, 2, 1, 4)
        .astype(ml_dtypes.bfloat16)
    ).reshape(B, HKV, 128, (SK // 128) * D)
    ones = np.ones((128, 1), ml_dtypes.bfloat16)

    in_maps = []
    for c in range(N_CORES):
        b = c // (N_CORES // B)
        part = c % (N_CORES // B)
        h0 = part * H_PER_CORE
        g0 = part * KV_PER_CORE
        in_maps.append(
            {
                "qT": qT_all[b, h0 : h0 + H_PER_CORE],
                "kT": kT_all[b, g0 : g0 + KV_PER_CORE],
                "vt": vt_all[b, g0 : g0 + KV_PER_CORE],
                "ones": ones,
            }
        )
    return in_maps


def gather_output(results):
    out = np.empty((B, SQ, H, D), np.float32)
    for c in range(N_CORES):
        b = c // (N_CORES // B)
        part = c % (N_CORES // B)
        h0 = part * H_PER_CORE
        # oT [n_heads, D, Sq] -> [Sq, n_heads, D]
        out[b, :, h0 : h0 + H_PER_CORE, :] = results[c]["oT"].transpose(2, 0, 1)
    return out


def run(q, kv, trace=False, **kwargs):
    nc = _get_nc()
    in_maps = make_in_maps(q, kv)
    res = run_bass_kernel_spmd(
        nc, in_maps, core_ids=list(range(N_CORES)), trace=trace, **kwargs
    )
    return gather_output(res.results), res


def kernel(q, kv):
    out, _ = run(q, kv, trace=False)
    return out
